# revision 14
# baseline (speedup 1.0000x reference)
"""MLA (DeepSeek-style multi-head latent attention) forward on 8 TRN2 NeuronCores.

Sharding: tensor-parallel over heads (16 heads -> 2 per core). The shared
q_down / ckv projections are replicated per core; per-head attention and the
output projection are sharded; partial wo outputs are summed on host.

Device layout is "feature-major" (features on SBUF partitions, sequence on the
free dim) throughout. Attention uses the prefill-optimal NON-absorbed form:
per-head K (128-dim nope) and V (128-dim) are materialized from the shared
latent once, so scores contract over 192 dims (not 576) and ctx over 128
(not 512). Scores come out k-major ([k, q]); softmax denominators are
accumulated on the vector engine and reduced across partitions on gpsimd,
keeping the tensor engine free for real matmuls.

All matmuls run in float32r (TF32: full speed on PE at free-dim >= 256).
Weights/host constants are pre-rounded to TF32 (RNE) on host; device-produced
matmul operands are written with float32r output dtype so the engines round.

Pipeline per core (S=2048, 4 seq-chunks of 512; heads h0=2c, h1=2c+1):
  Q:  q_downT = wq_a.T^T @ hidT (l-feature-major), 6 weight l-groups of 256;
      rms sum-of-squares via ACT Square + ones-matmul; partial wq_b
      contraction per l-group accumulated into a DRAM spill via DMA-add.
  C:  ckvT = wkv_a'.T^T @ hidT (RoPE interleave baked into the pe rows of
      wkv_a; the 64 pe rows duplicated to fill a 128-tile) -> DRAM spill.
  K:  rms-scale the latent (kv ln as per-partition scalar, rms via gpsimd
      partition-broadcast), materialize per-head k_nopeT (feature-major) and
      V (seq-major, both heads side by side), RoPE on k_pe.
  A:  per chunk: scale q by r_q, RoPE q_pe; per k-block, software-pipelined
      over both heads: scoresT -> exp (no max subtraction needed:
      |score*scale| <= ~4) -> causal mask on diagonal block (suffix-sliced
      matmuls skip fully-masked columns) -> ctxT accumulation in one PSUM
      bank; softmax denominator via DVE adds + one gpsimd partition
      all-reduce; normalization fused into the PSUM->SBUF ctx copy.
  W:  wo partial matmul -> DRAM outT.
Host: sum the 8 partial outT, transpose -> [1, S, HID].
"""

import numpy as np

S = 2048
HID = 2048
QLR = 1536
H_PER_CORE = 2
N_CORES = 8
NOPE = 128
ROPE = 64
VD = 128
KVL = 512
EPS = 1e-6
THETA = 10000.0
SCALE = float((NOPE + ROPE) ** -0.5)
NC_ = 4            # seq chunks
CW = 512           # chunk width
KB = S // 128      # 16 k-blocks


def _tf32_rne(a):
    a = np.ascontiguousarray(a, dtype=np.float32)
    u = a.view(np.uint32).astype(np.uint64)
    u = (u + 0xFFF + ((u >> 13) & 1)) & 0xFFFFE000
    return u.astype(np.uint32).view(np.float32)


def _build_program():
    import concourse.mybir as mybir
    import concourse.tile as tile
    import concourse.bass_isa as bass_isa
    from concourse import bacc

    f32 = mybir.dt.float32
    f32r = mybir.dt.float32r
    AF = mybir.ActivationFunctionType
    OP = mybir.AluOpType

    nc = bacc.Bacc("TRN2", target_bir_lowering=False)

    hid_t = nc.dram_tensor("hid_t", [HID, S], f32r, kind="ExternalInput")
    wqa_t = nc.dram_tensor("wqa_t", [HID, QLR], f32r, kind="ExternalInput")
    wqb_t = nc.dram_tensor("wqb_t", [QLR, 384], f32r, kind="ExternalInput")
    wkv_t = nc.dram_tensor("wkv_t", [HID, 640], f32r, kind="ExternalInput")
    kvln_d = nc.dram_tensor("kvln", [1, KVL], f32r, kind="ExternalInput")
    wukt_d = nc.dram_tensor("wukt", [H_PER_CORE, KVL, NOPE], f32r,
                            kind="ExternalInput")
    wuv2_d = nc.dram_tensor("wuv2", [KVL, H_PER_CORE * VD], f32r,
                            kind="ExternalInput")
    wo_t = nc.dram_tensor("wo_t", [H_PER_CORE * VD, HID], f32r, kind="ExternalInput")
    cos2_d = nc.dram_tensor("cos2", [128, S], f32, kind="ExternalInput")
    sin2n_d = nc.dram_tensor("sin2n", [128, S], f32, kind="ExternalInput")
    swapp_d = nc.dram_tensor("swapp", [128, 128], f32r, kind="ExternalInput")
    maskt_d = nc.dram_tensor("maskt", [128, 128], f32, kind="ExternalInput")
    out_t = nc.dram_tensor("out_t", [HID, S], f32, kind="ExternalOutput")

    with tile.TileContext(nc) as tc:
        with (
            tc.tile_pool(name="stats", bufs=1) as stats,
            tc.tile_pool(name="dram", bufs=1, space="DRAM") as dram,
        ):
            ones_p = stats.tile([128, 1], f32r)
            nc.vector.memset(ones_p.bitcast(f32), 1.0)
            ones_row = stats.tile([1, 128], f32r)
            nc.vector.memset(ones_row.bitcast(f32), 1.0)
            eps_sb = stats.tile([1, 1], f32)
            nc.vector.memset(eps_sb, EPS)
            ssq_sb = stats.tile([1, S], f32)
            rq_sb = stats.tile([1, S], f32)
            rqb_all = stats.tile([128, S], f32)

            qt_spill = dram.tile([128, 3, S], f32)
            ckv_spill = dram.tile([128, 5, S], f32r)

            # ================= Phases Q + C (hid resident) =================
            with tc.tile_pool(name="hidp", bufs=1) as hidp:
                hid_sb = hidp.tile([128, KB, S], f32r)
                for kt in range(KB):
                    nc.sync.dma_start(
                        hid_sb[:, kt, :], hid_t.ap()[128 * kt : 128 * (kt + 1), :]
                    )

                with (
                    tc.tile_pool(name="wqap", bufs=2) as wqap,
                    tc.tile_pool(name="qwork", bufs=2) as qwork,
                    tc.tile_pool(name="qpsum", bufs=2, space="PSUM") as qpsum,
                    tc.tile_pool(name="qpsum1", bufs=1, space="PSUM") as qpsum1,
                ):
                    for lb in range(6):
                        wqa_lb = wqap.tile([128, KB, 256], f32r, tag="wqa",
                                           name=f"wqa{lb}")
                        nc.sync.dma_start(
                            wqa_lb,
                            wqa_t.ap()[:, 256 * lb : 256 * (lb + 1)].rearrange(
                                "(kt p) m -> p kt m", p=128
                            ),
                        )
                        wqb_lb = wqap.tile([128, 2, 384], f32r, tag="wqb",
                                           name=f"wqb{lb}")
                        nc.sync.dma_start(
                            wqb_lb,
                            wqb_t.ap()[256 * lb : 256 * (lb + 1), :].rearrange(
                                "(lt p) m -> p lt m", p=128
                            ),
                        )
                        for c in range(NC_):
                            cs = slice(CW * c, CW * (c + 1))
                            qd_tiles = []
                            ps_ssq = qpsum1.tile([1, CW], f32, tag="ssq")
                            for lt in range(2):
                                ps_qd = qpsum.tile([128, CW], f32, tag="qd")
                                for kt in range(KB):
                                    nc.tensor.matmul(
                                        ps_qd,
                                        wqa_lb[:, kt, 128 * lt : 128 * (lt + 1)],
                                        hid_sb[:, kt, cs],
                                        start=(kt == 0),
                                        stop=(kt == KB - 1),
                                    )
                                qd = qwork.tile([128, CW], f32r, tag=f"qd_sb{lt}",
                                                name=f"qd{lt}")
                                nc.vector.tensor_copy(qd, ps_qd)
                                qd_tiles.append(qd)
                                sq = qwork.tile([128, CW], f32r, tag="sq")
                                nc.scalar.activation(sq, qd.bitcast(f32), AF.Square)
                                nc.tensor.matmul(
                                    ps_ssq, ones_p, sq, start=(lt == 0), stop=(lt == 1)
                                )
                            if lb == 0:
                                nc.vector.tensor_copy(ssq_sb[:, cs], ps_ssq)
                            else:
                                nc.vector.tensor_tensor(
                                    ssq_sb[:, cs], ssq_sb[:, cs], ps_ssq, OP.add
                                )
                            for dt in range(3):
                                ps_qt = qpsum.tile([128, CW], f32, tag="qt")
                                for lt in range(2):
                                    nc.tensor.matmul(
                                        ps_qt,
                                        wqb_lb[:, lt, 128 * dt : 128 * (dt + 1)],
                                        qd_tiles[lt],
                                        start=(lt == 0),
                                        stop=(lt == 1),
                                    )
                                qtb = qwork.tile([128, CW], f32, tag="qtb")
                                nc.vector.tensor_copy(qtb, ps_qt)
                                if lb == 0:
                                    nc.sync.dma_start(qt_spill[:, dt, cs], qtb)
                                else:
                                    nc.gpsimd.dma_start(
                                        qt_spill[:, dt, cs], qtb, accum_op=OP.add
                                    )

                    # r_q = 1/sqrt(ssq/QLR + eps); broadcast overlaps phase C
                    nc.scalar.activation(
                        rq_sb, ssq_sb, AF.Sqrt, scale=1.0 / QLR, bias=eps_sb
                    )
                    nc.vector.reciprocal_approx_fast(out=rq_sb, in_=rq_sb)
                    nc.gpsimd.partition_broadcast(rqb_all, rq_sb, channels=128)

                # ---- Phase C: ckvT (single resident wkv tile) ----
                with (
                    tc.tile_pool(name="cwork", bufs=2) as cwork,
                    tc.tile_pool(name="wkvp", bufs=1) as wkvp,
                    tc.tile_pool(name="cpsum", bufs=2, space="PSUM") as cpsum,
                ):
                    wkv_sb = wkvp.tile([128, KB, 640], f32r, tag="wkv")
                    for dt in range(5):
                        nc.sync.dma_start(
                            wkv_sb[:, :, 128 * dt : 128 * (dt + 1)],
                            wkv_t.ap()[:, 128 * dt : 128 * (dt + 1)].rearrange(
                                "(kt p) m -> p kt m", p=128
                            ),
                        )
                    for c in range(NC_):
                        cs = slice(CW * c, CW * (c + 1))
                        for dt in range(5):
                            ps_ck = cpsum.tile([128, CW], f32, tag="ck")
                            for kt in range(KB):
                                nc.tensor.matmul(
                                    ps_ck,
                                    wkv_sb[:, kt, 128 * dt : 128 * (dt + 1)],
                                    hid_sb[:, kt, cs],
                                    start=(kt == 0),
                                    stop=(kt == KB - 1),
                                )
                            ckb = cwork.tile([128, CW], f32r, tag="ckb")
                            nc.vector.tensor_copy(ckb, ps_ck)
                            nc.sync.dma_start(
                                ckv_spill[:, dt, cs], ckb
                            )

            # ============ late constants + persistent attention tensors ============
            with (
                tc.tile_pool(name="consts", bufs=1) as consts,
                tc.tile_pool(name="resid", bufs=1) as resid,
            ):
                kvln_sb = consts.tile([1, KVL], f32r)
                nc.sync.dma_start(kvln_sb, kvln_d.ap())
                wukt_sb = consts.tile([128, H_PER_CORE, 4, NOPE], f32r)
                nc.sync.dma_start(
                    wukt_sb, wukt_d.ap().rearrange("h (lt p) n -> p h lt n", p=128)
                )
                wuv2_sb = consts.tile([128, 4, H_PER_CORE * VD], f32r)
                nc.sync.dma_start(
                    wuv2_sb, wuv2_d.ap().rearrange("(lt p) v -> p lt v", p=128)
                )
                cos2_sb = consts.tile([128, S], f32)
                nc.sync.dma_start(cos2_sb, cos2_d.ap())
                sin2n_sb = consts.tile([128, S], f32)
                nc.sync.dma_start(sin2n_sb, sin2n_d.ap())
                swapp_sb = consts.tile([128, 128], f32r)
                nc.sync.dma_start(swapp_sb, swapp_d.ap())
                maskt_sb = consts.tile([128, 128], f32)
                nc.sync.dma_start(maskt_sb, maskt_d.ap())

                kpe = resid.tile([128, S], f32r)          # roped k_peT (2 head copies)
                kn_sb = resid.tile([128, H_PER_CORE, S], f32r)  # per-head k_nopeT
                v_sb = resid.tile([128, KB, H_PER_CORE * VD], f32r)  # V seq-major
                ctxa = resid.tile([128, H_PER_CORE, S], f32r)
                wo_sb = resid.tile([128, H_PER_CORE, HID], f32r)
                nc.sync.dma_start(
                    wo_sb, wo_t.ap().rearrange("(h p) m -> p h m", p=128)
                )

                # ===== Phase K: latent rms + per-head K/V materialization =====
                with (
                    tc.tile_pool(name="kwork", bufs=2) as kwork,
                    tc.tile_pool(name="kpsum", bufs=2, space="PSUM") as kpsum,
                    tc.tile_pool(name="kpsum1", bufs=1, space="PSUM") as kpsum1,
                ):
                    for c in range(NC_):
                        cs = slice(CW * c, CW * (c + 1))
                        ck = kwork.tile([128, 5, CW], f32r, tag="ck")
                        nc.sync.dma_start(ck, ckv_spill[:, :, cs])
                        ps_ssqk = kpsum1.tile([1, CW], f32, tag="ssqk")
                        for j in range(4):
                            sqk = kwork.tile([128, CW], f32r, tag="sqk")
                            nc.scalar.activation(
                                sqk, ck[:, j, :].bitcast(f32), AF.Square
                            )
                            nc.tensor.matmul(
                                ps_ssqk, ones_p, sqk, start=(j == 0), stop=(j == 3)
                            )
                        rk = kwork.tile([1, CW], f32, tag="rk")
                        nc.scalar.activation(
                            rk, ps_ssqk, AF.Sqrt, scale=1.0 / KVL, bias=eps_sb
                        )
                        nc.vector.reciprocal_approx_fast(out=rk, in_=rk)
                        rk_r = kwork.tile([1, CW], f32r, tag="rkr")
                        nc.vector.tensor_copy(rk_r, rk)
                        ksn_c = kwork.tile([128, 4, CW], f32r, tag="ksn")
                        for j in range(4):
                            ps_b = kpsum1.tile([128, CW], f32, tag="bc")
                            nc.tensor.matmul(
                                ps_b,
                                kvln_sb[0:1, 128 * j : 128 * (j + 1)],
                                rk_r,
                                start=True,
                                stop=True,
                            )
                            nc.vector.tensor_tensor(
                                ksn_c[:, j, :], ck[:, j, :].bitcast(f32), ps_b, OP.mult
                            )
                        # materialize per-head k_nopeT (feature-major)
                        for h in range(H_PER_CORE):
                            ps_k = kpsum.tile([128, CW], f32, tag="kn")
                            for lt in range(4):
                                nc.tensor.matmul(
                                    ps_k,
                                    wukt_sb[:, h, lt, :],
                                    ksn_c[:, lt, :],
                                    start=(lt == 0),
                                    stop=(lt == 3),
                                )
                            nc.vector.tensor_copy(kn_sb[:, h, cs], ps_k)
                        # materialize V (seq-major, both heads side by side)
                        for b in range(4):
                            ps_v = kpsum.tile([128, H_PER_CORE * VD], f32, tag="v")
                            for lt in range(4):
                                nc.tensor.matmul(
                                    ps_v,
                                    ksn_c[:, lt, 128 * b : 128 * (b + 1)],
                                    wuv2_sb[:, lt, :],
                                    start=(lt == 0),
                                    stop=(lt == 3),
                                )
                            nc.vector.tensor_copy(v_sb[:, 4 * c + b, :], ps_v)
                        # k_pe rope (both 64-row copies at once)
                        ps_sw = kpsum1.tile([128, CW], f32, tag="sw")
                        nc.tensor.matmul(
                            ps_sw, swapp_sb, ck[:, 4, :], start=True, stop=True
                        )
                        t1 = kwork.tile([128, CW], f32, tag="t1")
                        nc.vector.tensor_tensor(
                            t1, ck[:, 4, :].bitcast(f32), cos2_sb[:, cs], OP.mult
                        )
                        t2 = kwork.tile([128, CW], f32, tag="t2")
                        nc.vector.tensor_tensor(t2, ps_sw, sin2n_sb[:, cs], OP.mult)
                        nc.vector.tensor_tensor(kpe[:, cs], t1, t2, OP.add)

                # ================= Phase A: attention =================
                with (
                    tc.tile_pool(name="abig", bufs=2) as abig,
                    tc.tile_pool(name="asm", bufs=2) as asm,
                    tc.tile_pool(name="attp", bufs=3) as attp,
                    tc.tile_pool(name="aacc", bufs=1, space="PSUM") as aacc,
                    tc.tile_pool(name="ascore", bufs=2, space="PSUM") as ascore,
                    tc.tile_pool(name="ascr1", bufs=1, space="PSUM") as ascr1,
                    tc.tile_pool(name="amisc", bufs=1, space="PSUM") as amisc,
                ):
                    for c in range(NC_):
                        cs = slice(CW * c, CW * (c + 1))
                        qt = abig.tile([128, 3, CW], f32, tag="qt")
                        nc.sync.dma_start(qt, qt_spill[:, :, cs])
                        qtr = abig.tile([128, 3, CW], f32r, tag="qtr")
                        for dt in range(3):
                            nc.vector.tensor_tensor(
                                qtr[:, dt, :], qt[:, dt, :], rqb_all[:, cs], OP.mult
                            )
                        # rope q_pe (both heads stacked)
                        ps_sw = amisc.tile([128, CW], f32, tag="misc", name="ps_sw")
                        nc.tensor.matmul(
                            ps_sw, swapp_sb, qtr[:, 2, :], start=True, stop=True
                        )
                        t1 = asm.tile([128, CW], f32, tag="t1")
                        nc.vector.tensor_tensor(
                            t1, qtr[:, 2, :].bitcast(f32), cos2_sb[:, cs], OP.mult
                        )
                        t2 = asm.tile([128, CW], f32, tag="t2")
                        nc.vector.tensor_tensor(t2, ps_sw, sin2n_sb[:, cs], OP.mult)
                        qspe = asm.tile([128, CW], f32r, tag="qspe")
                        nc.vector.tensor_tensor(qspe, t1, t2, OP.add)

                        nj = 4 * c + 4
                        ps_ctx = [
                            aacc.tile([128, CW], f32, tag=f"ctx{h}",
                                      name=f"ps_ctx{h}")
                            for h in range(H_PER_CORE)
                        ]
                        ps_sum = [
                            aacc.tile([1, CW], f32, tag=f"sum{h}",
                                      name=f"ps_sum{h}")
                            for h in range(H_PER_CORE)
                        ]
                        # software pipeline: scores(j) on PE, then ctx(j-1);
                        # exp(j) on ACT overlaps ctx(j-1)+scores(j+1) on PE.
                        atts = [[None, None] for _ in range(nj)]
                        offs = [0 if j < 4 * c else 128 * (j - 4 * c)
                                for j in range(nj)]

                        def emit_scores(j):
                            off = offs[j]
                            for h in range(H_PER_CORE):
                                pool = ascore if h == 0 else ascr1
                                ps_s = pool.tile([128, CW], f32, tag=f"sc{h}",
                                                 name=f"ps_s{h}")
                                nc.tensor.matmul(
                                    ps_s[:, off:],
                                    kn_sb[:, h, 128 * j : 128 * (j + 1)],
                                    qtr[:, h, off:],
                                    start=True,
                                    stop=False,
                                )
                                nc.tensor.matmul(
                                    ps_s[:, off:],
                                    kpe[64 * h : 64 * (h + 1),
                                        128 * j : 128 * (j + 1)],
                                    qspe[64 * h : 64 * (h + 1), off:],
                                    start=False,
                                    stop=True,
                                )
                                att = attp.tile([128, CW], f32r, tag=f"att{h}",
                                                name=f"att{h}")
                                nc.scalar.activation(
                                    att[:, off:], ps_s[:, off:], AF.Exp, scale=SCALE
                                )
                                if j >= 4 * c:
                                    nc.vector.tensor_tensor(
                                        att[:, off : off + 128],
                                        att[:, off : off + 128].bitcast(f32),
                                        maskt_sb,
                                        OP.mult,
                                    )
                                atts[j][h] = att

                        def emit_ctx(j):
                            off = offs[j]
                            for h in range(H_PER_CORE):
                                nc.tensor.matmul(
                                    ps_ctx[h][:, off:],
                                    v_sb[:, j, VD * h : VD * (h + 1)],
                                    atts[j][h][:, off:],
                                    start=(j == 0),
                                    stop=(j == nj - 1),
                                )
                                nc.tensor.matmul(
                                    ps_sum[h][:, off:],
                                    ones_p,
                                    atts[j][h][:, off:],
                                    start=(j == 0),
                                    stop=(j == nj - 1),
                                )

                        emit_scores(0)
                        for j in range(1, nj):
                            emit_scores(j)
                            emit_ctx(j - 1)
                        emit_ctx(nj - 1)

                        for h in range(H_PER_CORE):
                            s_sb = asm.tile([1, CW], f32, tag="ssb")
                            nc.vector.tensor_copy(s_sb, ps_sum[h])
                            nc.vector.reciprocal_approx_fast(out=s_sb, in_=s_sb)
                            rs_r = asm.tile([1, CW], f32r, tag="rsr")
                            nc.vector.tensor_copy(rs_r, s_sb)
                            ps_rb2 = amisc.tile([128, CW], f32, tag="misc",
                                                name="ps_rb2")
                            nc.tensor.matmul(
                                ps_rb2, ones_row, rs_r, start=True, stop=True
                            )
                            rsb = asm.tile([128, CW], f32, tag="rsb")
                            nc.vector.tensor_copy(rsb, ps_rb2)
                            nc.vector.tensor_tensor(
                                ctxa[:, h, cs], ps_ctx[h], rsb, OP.mult
                            )

                # ================= Phase W: output projection =================
                with (
                    tc.tile_pool(name="obp", bufs=3) as obp,
                    tc.tile_pool(name="wpsum", bufs=2, space="PSUM") as wpsum,
                ):
                    for c in range(NC_):
                        cs = slice(CW * c, CW * (c + 1))
                        for ht in range(KB):
                            ps_o = wpsum.tile([128, CW], f32, tag="o")
                            for h in range(H_PER_CORE):
                                nc.tensor.matmul(
                                    ps_o,
                                    wo_sb[:, h, 128 * ht : 128 * (ht + 1)],
                                    ctxa[:, h, cs],
                                    start=(h == 0),
                                    stop=(h == H_PER_CORE - 1),
                                )
                            ob = obp.tile([128, CW], f32, tag="ob")
                            nc.vector.tensor_copy(ob, ps_o)
                            nc.sync.dma_start(
                                out_t.ap()[128 * ht : 128 * (ht + 1), cs], ob
                            )

    nc.finalize()
    return nc


_PROGRAM = None


def _get_program():
    global _PROGRAM
    if _PROGRAM is None:
        _PROGRAM = _build_program()
    return _PROGRAM


def _host_inputs(hidden_states, position_ids, wq_a, q_a_ln_w, wq_b, wkv_a,
                 kv_a_ln_w, wkv_b, wo):
    """Build the 8 per-core input maps."""
    hs = np.asarray(hidden_states, np.float32)[0]          # [S, HID]
    pos = np.asarray(position_ids)[0].astype(np.int64)     # [S]

    # rope tables (fp32, matching the reference)
    inv_freq = (1.0 / (THETA ** (np.arange(0, ROPE, 2, dtype=np.float32) / ROPE))).astype(np.float32)
    t = pos.astype(np.float32)
    freqs = np.outer(t, inv_freq).astype(np.float32)       # [S, 32]
    emb = np.concatenate([freqs, freqs], -1)               # [S, 64]
    cos = np.cos(emb).astype(np.float32)
    sin = np.sin(emb).astype(np.float32)
    cosT = np.ascontiguousarray(cos.T)                     # [64, S]
    sinT = np.ascontiguousarray(sin.T)
    sinTn = sinT.copy()
    sinTn[:32] = -sinTn[:32]                               # fold rotate_half sign
    cos2 = np.concatenate([cosT, cosT], 0)                 # [128, S]
    sin2n = np.concatenate([sinTn, sinTn], 0)

    perm = np.concatenate([np.arange(0, ROPE, 2), np.arange(1, ROPE, 2)])  # interleave

    # swap-halves permutation matrix (two independent 64 blocks)
    swapp = np.zeros((128, 128), np.float32)
    for m in range(128):
        base = (m // 64) * 64
        i = m % 64
        swapp[base + (i + 32) % 64, m] = 1.0

    maskt = np.triu(np.ones((128, 128), np.float32))

    wq_b = np.asarray(wq_b, np.float32) * np.asarray(q_a_ln_w, np.float32)[None, :]
    kvb = np.asarray(wkv_b, np.float32).reshape(16, NOPE + VD, KVL)
    wkv_a = np.asarray(wkv_a, np.float32)
    wkv_rows = np.concatenate(
        [wkv_a[:KVL], wkv_a[KVL:][perm], wkv_a[KVL:][perm]], 0
    )                                                      # [640, HID]

    shared = {
        "hid_t": _tf32_rne(hs.T),
        "wqa_t": _tf32_rne(np.asarray(wq_a, np.float32).T),
        "wkv_t": _tf32_rne(wkv_rows.T),
        "kvln": _tf32_rne(np.asarray(kv_a_ln_w, np.float32)[None, :]),
        "cos2": cos2, "sin2n": sin2n,
        "swapp": _tf32_rne(swapp), "maskt": maskt,
    }

    wo = np.asarray(wo, np.float32)
    in_maps = []
    for core in range(N_CORES):
        h0 = H_PER_CORE * core
        blocks = []
        pe_rows = []
        for h in (h0, h0 + 1):
            blk = wq_b[192 * h : 192 * (h + 1)]
            blocks.append(blk[:NOPE])
            pe_rows.append(blk[NOPE:][perm])
        wqb_re = np.concatenate(blocks + pe_rows, 0)       # [384, QLR]
        wukt = np.stack(
            [np.ascontiguousarray(kvb[h, :NOPE, :].T) for h in (h0, h0 + 1)]
        )                                                  # [2, 512, 128]
        wuv2 = np.concatenate(
            [kvb[h, NOPE:, :].T for h in (h0, h0 + 1)], axis=1
        )                                                  # [512, 256]
        wo_c = np.ascontiguousarray(wo[:, VD * h0 : VD * (h0 + 2)].T)   # [256, HID]
        in_maps.append({
            **shared,
            "wqb_t": _tf32_rne(wqb_re.T),
            "wukt": _tf32_rne(wukt),
            "wuv2": _tf32_rne(np.ascontiguousarray(wuv2)),
            "wo_t": _tf32_rne(wo_c),
        })
    return in_maps


def kernel(**inputs):
    from concourse.bass_utils import run_bass_kernel_spmd

    nc = _get_program()
    in_maps = _host_inputs(**inputs)
    res = run_bass_kernel_spmd(nc, in_maps, core_ids=list(range(N_CORES)))
    acc = None
    for r in res.results:
        o = r["out_t"]
        acc = o.copy() if acc is None else acc + o
    out = np.ascontiguousarray(acc.T)[None]                # [1, S, HID]
    return out.astype(np.float32)


# revision 18
# speedup vs baseline: 3.6817x; 3.6817x over previous
"""MLA (DeepSeek-style multi-head latent attention) forward on 8 TRN2 NeuronCores.

Sharding: tensor-parallel over heads (16 heads -> 2 per core). The shared
q_down / ckv projections are replicated per core; per-head attention and the
output projection are sharded; partial wo outputs are summed on host.

Device layout is "feature-major" (features on SBUF partitions, sequence on the
free dim) throughout. Attention uses the prefill-optimal NON-absorbed form:
per-head K (128-dim nope) and V (128-dim) are materialized from the shared
latent once, so scores contract over 192 dims (not 576) and ctx over 128
(not 512). Scores come out k-major ([k, q]); softmax normalization over k is
done with ones-matmuls on the tensor engine.

The replicated projections (q_down / ckv) run with bf16 inputs and weights
(hid/wq_a/wq_b/wkv_a rounded on host) -- same PE rate as fp32r but half the
DMA -- accumulating in fp32 PSUM. Everything downstream (attention operands)
stays float32r (TF32); the wo partials are written back in bf16 and summed
in fp32 on host.

Pipeline per core (S=2048, 4 seq-chunks of 512; heads h0=2c, h1=2c+1):
  Q:  q_downT = wq_a.T^T @ hidT (l-feature-major), 6 weight l-groups of 256,
      double-buffered; rms sum-of-squares via ACT Square read straight from
      PSUM + ones-matmul, emitted after both l-tiles' matmul groups so the
      PE never waits on the DVE/ACT chain; partial wq_b contraction per
      l-group accumulated into a DRAM spill via DMA-add.
  C:  ckvT = wkv_a'.T^T @ hidT (RoPE interleave baked into the pe rows of
      wkv_a; the 64 pe rows duplicated to fill a 128-tile) -> DRAM spill.
      wkv stays resident from mid-phase-Q so the transition never stalls.
  K:  two-stage software pipeline per chunk: rms-scale the latent (kv ln
      folded into the broadcast matmul), then materialize per-head k_nopeT
      (feature-major) and V (seq-major, both heads side by side) for the
      previous chunk while the next chunk's rms chain runs; RoPE on k_pe.
  A:  all q-side prep (r_q scale, RoPE) for all 4 chunks upfront; per
      k-block, software-pipelined over both heads: scoresT -> exp (no max
      subtraction needed: |score*scale| <= ~4) -> causal mask on diagonal
      block (suffix-sliced matmuls skip fully-masked columns) -> ctxT +
      softmax denominator accumulation; per-head epilogues deferred into the
      next chunk's score stream.
  W:  wo partial matmul -> bf16 DRAM outT (half the write traffic keeps the
      tail compute-bound).
Host: sum the 8 partial outT in fp32, transpose -> [1, S, HID].
"""

import numpy as np

S = 2048
HID = 2048
QLR = 1536
H_PER_CORE = 2
N_CORES = 8
NOPE = 128
ROPE = 64
VD = 128
KVL = 512
EPS = 1e-6
THETA = 10000.0
SCALE = float((NOPE + ROPE) ** -0.5)
NC_ = 4            # seq chunks
CW = 512           # chunk width
KB = S // 128      # 16 k-blocks


def _tf32_rne(a):
    a = np.ascontiguousarray(a, dtype=np.float32)
    u = a.view(np.uint32).astype(np.uint64)
    u = (u + 0xFFF + ((u >> 13) & 1)) & 0xFFFFE000
    return u.astype(np.uint32).view(np.float32)


def _bf16(a):
    import ml_dtypes
    return np.ascontiguousarray(np.asarray(a, np.float32)).astype(ml_dtypes.bfloat16)


def _build_program():
    import concourse.mybir as mybir
    import concourse.tile as tile
    from concourse import bacc

    f32 = mybir.dt.float32
    f32r = mybir.dt.float32r
    bf16 = mybir.dt.bfloat16
    AF = mybir.ActivationFunctionType
    OP = mybir.AluOpType

    nc = bacc.Bacc("TRN2", target_bir_lowering=False)

    hid_t = nc.dram_tensor("hid_t", [HID, S], bf16, kind="ExternalInput")
    wqa_t = nc.dram_tensor("wqa_t", [HID, QLR], bf16, kind="ExternalInput")
    wqb_t = nc.dram_tensor("wqb_t", [QLR, 384], bf16, kind="ExternalInput")
    wkv_t = nc.dram_tensor("wkv_t", [HID, 640], bf16, kind="ExternalInput")
    kvln_d = nc.dram_tensor("kvln", [1, KVL], f32r, kind="ExternalInput")
    wukt_d = nc.dram_tensor("wukt", [H_PER_CORE, KVL, NOPE], f32r,
                            kind="ExternalInput")
    wuv2_d = nc.dram_tensor("wuv2", [KVL, H_PER_CORE * VD], f32r,
                            kind="ExternalInput")
    wo_t = nc.dram_tensor("wo_t", [H_PER_CORE * VD, HID], f32r, kind="ExternalInput")
    cos2_d = nc.dram_tensor("cos2", [128, S], f32, kind="ExternalInput")
    sin2n_d = nc.dram_tensor("sin2n", [128, S], f32, kind="ExternalInput")
    swapp_d = nc.dram_tensor("swapp", [128, 128], f32r, kind="ExternalInput")
    maskt_d = nc.dram_tensor("maskt", [128, 128], f32, kind="ExternalInput")
    out_t = nc.dram_tensor("out_t", [HID, S], bf16, kind="ExternalOutput")

    with tile.TileContext(nc) as tc:
        with (
            tc.tile_pool(name="stats", bufs=1) as stats,
            tc.tile_pool(name="dram", bufs=1, space="DRAM") as dram,
        ):
            ones_p = stats.tile([128, 1], f32r)
            nc.vector.memset(ones_p.bitcast(f32), 1.0)
            ones_row = stats.tile([1, 128], f32r)
            nc.vector.memset(ones_row.bitcast(f32), 1.0)
            eps_sb = stats.tile([1, 1], f32)
            nc.vector.memset(eps_sb, EPS)
            ssq_sb = stats.tile([1, S], f32)
            rq_sb = stats.tile([1, S], f32)
            rqb_all = stats.tile([128, S], f32)

            qt_spill = dram.tile([128, 3, S], f32)
            ckv_spill = dram.tile([128, 5, S], f32r)

            # ================= Phases Q + C (hid resident) =================
            with (
                tc.tile_pool(name="hidp", bufs=1) as hidp,
                tc.tile_pool(name="wkvp", bufs=1) as wkvp,
            ):
                hid_sb = hidp.tile([128, KB, S], bf16)
                for kt in range(KB):
                    nc.sync.dma_start(
                        hid_sb[:, kt, :], hid_t.ap()[128 * kt : 128 * (kt + 1), :]
                    )
                wkv_sb = wkvp.tile([128, KB, 640], bf16, tag="wkv")

                with (
                    tc.tile_pool(name="wqap", bufs=2) as wqap,
                    tc.tile_pool(name="qwork", bufs=2) as qwork,
                    tc.tile_pool(name="qpsum", bufs=2, space="PSUM") as qpsum,
                    tc.tile_pool(name="qpsum1", bufs=1, space="PSUM") as qpsum1,
                ):
                    for lb in range(6):
                        wqa_lb = wqap.tile([128, KB, 256], bf16, tag="wqa",
                                           name=f"wqa{lb}")
                        nc.sync.dma_start(
                            wqa_lb,
                            wqa_t.ap()[:, 256 * lb : 256 * (lb + 1)].rearrange(
                                "(kt p) m -> p kt m", p=128
                            ),
                        )
                        wqb_lb = wqap.tile([128, 2, 384], bf16, tag="wqb",
                                           name=f"wqb{lb}")
                        nc.sync.dma_start(
                            wqb_lb,
                            wqb_t.ap()[256 * lb : 256 * (lb + 1), :].rearrange(
                                "(lt p) m -> p lt m", p=128
                            ),
                        )
                        if lb == 1:
                            # prefetch wkv while phase Q computes
                            for dt in range(5):
                                nc.sync.dma_start(
                                    wkv_sb[:, :, 128 * dt : 128 * (dt + 1)],
                                    wkv_t.ap()[:, 128 * dt : 128 * (dt + 1)]
                                    .rearrange("(kt p) m -> p kt m", p=128),
                                )
                        for c in range(NC_):
                            cs = slice(CW * c, CW * (c + 1))
                            qd_tiles = []
                            ps_qds = []
                            for lt in range(2):
                                ps_qd = qpsum.tile([128, CW], f32, tag=f"qd{lt}",
                                                   name=f"ps_qd{lt}")
                                for kt in range(KB):
                                    nc.tensor.matmul(
                                        ps_qd,
                                        wqa_lb[:, kt, 128 * lt : 128 * (lt + 1)],
                                        hid_sb[:, kt, cs],
                                        start=(kt == 0),
                                        stop=(kt == KB - 1),
                                    )
                                qd = qwork.tile([128, CW], bf16, tag=f"qd_sb{lt}",
                                                name=f"qd{lt}")
                                nc.vector.tensor_copy(qd, ps_qd)
                                qd_tiles.append(qd)
                                ps_qds.append(ps_qd)
                            # squares read PSUM directly; emitted after both
                            # qd groups so the PE never waits on ACT.
                            ps_ssq = qpsum1.tile([1, CW], f32, tag="ssq")
                            for lt in range(2):
                                sq = qwork.tile([128, CW], f32r, tag="sq")
                                nc.scalar.activation(sq, ps_qds[lt], AF.Square)
                                nc.tensor.matmul(
                                    ps_ssq, ones_p, sq, start=(lt == 0),
                                    stop=(lt == 1)
                                )
                            for dt in range(3):
                                ps_qt = qpsum.tile([128, CW], f32, tag="qt")
                                for lt in range(2):
                                    nc.tensor.matmul(
                                        ps_qt,
                                        wqb_lb[:, lt, 128 * dt : 128 * (dt + 1)],
                                        qd_tiles[lt],
                                        start=(lt == 0),
                                        stop=(lt == 1),
                                    )
                                qtb = qwork.tile([128, CW], f32, tag="qtb")
                                nc.vector.tensor_copy(qtb, ps_qt)
                                if lb == 0:
                                    nc.sync.dma_start(qt_spill[:, dt, cs], qtb)
                                else:
                                    nc.gpsimd.dma_start(
                                        qt_spill[:, dt, cs], qtb, accum_op=OP.add
                                    )
                            if lb == 0:
                                nc.vector.tensor_copy(ssq_sb[:, cs], ps_ssq)
                            else:
                                nc.vector.tensor_tensor(
                                    ssq_sb[:, cs], ssq_sb[:, cs], ps_ssq, OP.add
                                )

                    # r_q = 1/sqrt(ssq/QLR + eps); broadcast overlaps phase C
                    nc.scalar.activation(
                        rq_sb, ssq_sb, AF.Sqrt, scale=1.0 / QLR, bias=eps_sb
                    )
                    nc.vector.reciprocal_approx_fast(out=rq_sb, in_=rq_sb)
                    nc.gpsimd.partition_broadcast(rqb_all, rq_sb, channels=128)

                # ---- Phase C: ckvT (wkv already resident) ----
                with (
                    tc.tile_pool(name="cwork", bufs=2) as cwork,
                    tc.tile_pool(name="cpsum", bufs=2, space="PSUM") as cpsum,
                ):
                    for c in range(NC_):
                        cs = slice(CW * c, CW * (c + 1))
                        for dt in range(5):
                            ps_ck = cpsum.tile([128, CW], f32, tag="ck")
                            for kt in range(KB):
                                nc.tensor.matmul(
                                    ps_ck,
                                    wkv_sb[:, kt, 128 * dt : 128 * (dt + 1)],
                                    hid_sb[:, kt, cs],
                                    start=(kt == 0),
                                    stop=(kt == KB - 1),
                                )
                            ckb = cwork.tile([128, CW], f32r, tag="ckb")
                            nc.vector.tensor_copy(ckb, ps_ck)
                            nc.sync.dma_start(
                                ckv_spill[:, dt, cs], ckb
                            )

            # ============ late constants + persistent attention tensors ============
            with (
                tc.tile_pool(name="consts", bufs=1) as consts,
                tc.tile_pool(name="resid", bufs=1) as resid,
            ):
                kvln_sb = consts.tile([1, KVL], f32r)
                nc.sync.dma_start(kvln_sb, kvln_d.ap())
                wukt_sb = consts.tile([128, H_PER_CORE, 4, NOPE], f32r)
                nc.sync.dma_start(
                    wukt_sb, wukt_d.ap().rearrange("h (lt p) n -> p h lt n", p=128)
                )
                wuv2_sb = consts.tile([128, 4, H_PER_CORE * VD], f32r)
                nc.sync.dma_start(
                    wuv2_sb, wuv2_d.ap().rearrange("(lt p) v -> p lt v", p=128)
                )
                cos2_sb = consts.tile([128, S], f32)
                nc.sync.dma_start(cos2_sb, cos2_d.ap())
                sin2n_sb = consts.tile([128, S], f32)
                nc.sync.dma_start(sin2n_sb, sin2n_d.ap())
                swapp_sb = consts.tile([128, 128], f32r)
                nc.sync.dma_start(swapp_sb, swapp_d.ap())
                maskt_sb = consts.tile([128, 128], f32)
                nc.sync.dma_start(maskt_sb, maskt_d.ap())

                kpe = resid.tile([128, S], f32r)          # roped k_peT (2 head copies)
                kn_sb = resid.tile([128, H_PER_CORE, S], f32r)  # per-head k_nopeT
                v_sb = resid.tile([128, KB, H_PER_CORE * VD], f32r)  # V seq-major
                ctxa = resid.tile([128, H_PER_CORE, S], f32r)
                wo_sb = resid.tile([128, H_PER_CORE, HID], f32r)
                nc.sync.dma_start(
                    wo_sb, wo_t.ap().rearrange("(h p) m -> p h m", p=128)
                )

                # ===== Phase K: latent rms + per-head K/V materialization =====
                # Two-stage pipeline: rms chain of chunk c overlaps the
                # materialization matmuls of chunk c-1 on the PE.
                with (
                    tc.tile_pool(name="kwork", bufs=2) as kwork,
                    tc.tile_pool(name="kpsum", bufs=2, space="PSUM") as kpsum,
                    tc.tile_pool(name="kpsum1", bufs=1, space="PSUM") as kpsum1,
                ):
                    ksn_tiles = [None] * NC_
                    ck_tiles = [None] * NC_
                    rk_tiles = [None] * NC_

                    def emit_rms_a(c):
                        cs = slice(CW * c, CW * (c + 1))
                        ck = kwork.tile([128, 5, CW], f32r, tag="ck",
                                        name=f"ck{c}")
                        nc.sync.dma_start(ck, ckv_spill[:, :, cs])
                        ps_ssqk = kpsum1.tile([1, CW], f32, tag="ssqk")
                        for j in range(4):
                            sqk = kwork.tile([128, CW], f32r, tag="sqk")
                            nc.scalar.activation(
                                sqk, ck[:, j, :].bitcast(f32), AF.Square
                            )
                            nc.tensor.matmul(
                                ps_ssqk, ones_p, sqk, start=(j == 0), stop=(j == 3)
                            )
                        rk = kwork.tile([1, CW], f32, tag="rk")
                        nc.scalar.activation(
                            rk, ps_ssqk, AF.Sqrt, scale=1.0 / KVL, bias=eps_sb
                        )
                        nc.vector.reciprocal_approx_fast(out=rk, in_=rk)
                        rk_r = kwork.tile([1, CW], f32r, tag="rkr")
                        nc.vector.tensor_copy(rk_r, rk)
                        ck_tiles[c] = ck
                        rk_tiles[c] = rk_r

                    def emit_rms_b(c):
                        ck = ck_tiles[c]
                        rk_r = rk_tiles[c]
                        ksn_c = kwork.tile([128, 4, CW], f32r, tag="ksn",
                                           name=f"ksn{c}")
                        for j in range(4):
                            ps_b = kpsum1.tile([128, CW], f32, tag="bc")
                            nc.tensor.matmul(
                                ps_b,
                                kvln_sb[0:1, 128 * j : 128 * (j + 1)],
                                rk_r,
                                start=True,
                                stop=True,
                            )
                            nc.vector.tensor_tensor(
                                ksn_c[:, j, :], ck[:, j, :].bitcast(f32), ps_b,
                                OP.mult
                            )
                        ksn_tiles[c] = ksn_c

                    def emit_mat(c):
                        cs = slice(CW * c, CW * (c + 1))
                        ksn_c = ksn_tiles[c]
                        ck = ck_tiles[c]
                        for h in range(H_PER_CORE):
                            ps_k = kpsum.tile([128, CW], f32, tag="kn")
                            for lt in range(4):
                                nc.tensor.matmul(
                                    ps_k,
                                    wukt_sb[:, h, lt, :],
                                    ksn_c[:, lt, :],
                                    start=(lt == 0),
                                    stop=(lt == 3),
                                )
                            nc.vector.tensor_copy(kn_sb[:, h, cs], ps_k)
                        for b in range(4):
                            ps_v = kpsum.tile([128, H_PER_CORE * VD], f32, tag="v")
                            for lt in range(4):
                                nc.tensor.matmul(
                                    ps_v,
                                    ksn_c[:, lt, 128 * b : 128 * (b + 1)],
                                    wuv2_sb[:, lt, :],
                                    start=(lt == 0),
                                    stop=(lt == 3),
                                )
                            nc.vector.tensor_copy(v_sb[:, 4 * c + b, :], ps_v)
                        # k_pe rope (both 64-row copies at once)
                        ps_sw = kpsum1.tile([128, CW], f32, tag="sw")
                        nc.tensor.matmul(
                            ps_sw, swapp_sb, ck[:, 4, :], start=True, stop=True
                        )
                        t1 = kwork.tile([128, CW], f32, tag="t1")
                        nc.vector.tensor_tensor(
                            t1, ck[:, 4, :].bitcast(f32), cos2_sb[:, cs], OP.mult
                        )
                        t2 = kwork.tile([128, CW], f32, tag="t2")
                        nc.vector.tensor_tensor(t2, ps_sw, sin2n_sb[:, cs], OP.mult)
                        nc.vector.tensor_tensor(kpe[:, cs], t1, t2, OP.add)

                    emit_rms_a(0)
                    emit_rms_b(0)
                    for c in range(1, NC_):
                        emit_rms_a(c)
                        emit_mat(c - 1)
                        emit_rms_b(c)
                    emit_mat(NC_ - 1)

                # ================= Phase A: attention =================
                with (
                    tc.tile_pool(name="abig", bufs=1) as abig,
                    tc.tile_pool(name="asm", bufs=2) as asm,
                    tc.tile_pool(name="attp", bufs=3) as attp,
                    tc.tile_pool(name="aacc", bufs=1, space="PSUM") as aacc,
                    tc.tile_pool(name="ascore", bufs=2, space="PSUM") as ascore,
                    tc.tile_pool(name="ascr1", bufs=1, space="PSUM") as ascr1,
                    tc.tile_pool(name="amisc", bufs=1, space="PSUM") as amisc,
                ):
                    # ---- all q-side prep upfront (r_q scale in place) ----
                    qt_all = abig.tile([128, 3, S], f32, tag="qt")
                    nc.sync.dma_start(qt_all, qt_spill[:, :, :])
                    qtr = qt_all.bitcast(f32r)
                    for c in range(NC_):
                        cs = slice(CW * c, CW * (c + 1))
                        for dt in range(3):
                            nc.vector.tensor_tensor(
                                qtr[:, dt, cs], qt_all[:, dt, cs], rqb_all[:, cs],
                                OP.mult
                            )
                    qspe = abig.tile([128, S], f32r, tag="qspe")
                    for c in range(NC_):
                        cs = slice(CW * c, CW * (c + 1))
                        ps_sw = amisc.tile([128, CW], f32, tag="misc",
                                           name="ps_sw")
                        nc.tensor.matmul(
                            ps_sw, swapp_sb, qtr[:, 2, cs], start=True, stop=True
                        )
                        t1 = asm.tile([128, CW], f32, tag="t1")
                        nc.vector.tensor_tensor(
                            t1, qtr[:, 2, cs].bitcast(f32), cos2_sb[:, cs], OP.mult
                        )
                        t2 = asm.tile([128, CW], f32, tag="t2")
                        nc.vector.tensor_tensor(t2, ps_sw, sin2n_sb[:, cs], OP.mult)
                        nc.vector.tensor_tensor(qspe[:, cs], t1, t2, OP.add)

                    pending_epilogue = [None]

                    for c in range(NC_):
                        cs = slice(CW * c, CW * (c + 1))
                        nj = 4 * c + 4
                        ps_ctx = [
                            aacc.tile([128, CW], f32, tag=f"ctx{h}",
                                      name=f"ps_ctx{h}")
                            for h in range(H_PER_CORE)
                        ]
                        ps_sum = [
                            aacc.tile([1, CW], f32, tag=f"sum{h}",
                                      name=f"ps_sum{h}")
                            for h in range(H_PER_CORE)
                        ]
                        # software pipeline: scores(j) on PE, then ctx(j-1);
                        # exp(j) on ACT overlaps ctx(j-1)+scores(j+1) on PE.
                        atts = [[None, None] for _ in range(nj)]
                        offs = [0 if j < 4 * c else 128 * (j - 4 * c)
                                for j in range(nj)]

                        def emit_scores(j, c=c, nj=nj, offs=offs, atts=atts):
                            off = offs[j]
                            q0 = CW * c + off
                            qs = slice(q0, CW * (c + 1))
                            for h in range(H_PER_CORE):
                                pool = ascore if h == 0 else ascr1
                                ps_s = pool.tile([128, CW], f32, tag=f"sc{h}",
                                                 name=f"ps_s{h}")
                                nc.tensor.matmul(
                                    ps_s[:, off:],
                                    kn_sb[:, h, 128 * j : 128 * (j + 1)],
                                    qtr[:, h, qs],
                                    start=True,
                                    stop=False,
                                )
                                nc.tensor.matmul(
                                    ps_s[:, off:],
                                    kpe[64 * h : 64 * (h + 1),
                                        128 * j : 128 * (j + 1)],
                                    qspe[64 * h : 64 * (h + 1), qs],
                                    start=False,
                                    stop=True,
                                )
                                att = attp.tile([128, CW], f32r, tag=f"att{h}",
                                                name=f"att{h}")
                                nc.scalar.activation(
                                    att[:, off:], ps_s[:, off:], AF.Exp, scale=SCALE
                                )
                                if j >= 4 * c:
                                    nc.vector.tensor_tensor(
                                        att[:, off : off + 128],
                                        att[:, off : off + 128].bitcast(f32),
                                        maskt_sb,
                                        OP.mult,
                                    )
                                atts[j][h] = att

                        def emit_ctx(j, c=c, nj=nj, offs=offs, atts=atts,
                                     ps_ctx=ps_ctx, ps_sum=ps_sum):
                            off = offs[j]
                            for h in range(H_PER_CORE):
                                nc.tensor.matmul(
                                    ps_ctx[h][:, off:],
                                    v_sb[:, j, VD * h : VD * (h + 1)],
                                    atts[j][h][:, off:],
                                    start=(j == 0),
                                    stop=(j == nj - 1),
                                )
                                nc.tensor.matmul(
                                    ps_sum[h][:, off:],
                                    ones_p,
                                    atts[j][h][:, off:],
                                    start=(j == 0),
                                    stop=(j == nj - 1),
                                )

                        emit_scores(0)
                        emit_scores(1)
                        if pending_epilogue[0] is not None:
                            pending_epilogue[0]()
                        emit_ctx(0)
                        for j in range(2, nj):
                            emit_scores(j)
                            emit_ctx(j - 1)
                        emit_ctx(nj - 1)

                        def epilogue(c=c, cs=cs, ps_ctx=ps_ctx, ps_sum=ps_sum):
                            for h in range(H_PER_CORE):
                                s_sb = asm.tile([1, CW], f32, tag="ssb")
                                nc.vector.tensor_copy(s_sb, ps_sum[h])
                                nc.vector.reciprocal_approx_fast(
                                    out=s_sb, in_=s_sb
                                )
                                rs_r = asm.tile([1, CW], f32r, tag="rsr")
                                nc.vector.tensor_copy(rs_r, s_sb)
                                ps_rb2 = amisc.tile([128, CW], f32, tag="misc",
                                                    name="ps_rb2")
                                nc.tensor.matmul(
                                    ps_rb2, ones_row, rs_r, start=True, stop=True
                                )
                                rsb = asm.tile([128, CW], f32, tag="rsb")
                                nc.vector.tensor_copy(rsb, ps_rb2)
                                nc.vector.tensor_tensor(
                                    ctxa[:, h, cs], ps_ctx[h], rsb, OP.mult
                                )

                        pending_epilogue[0] = epilogue
                    pending_epilogue[0]()

                # ================= Phase W: output projection =================
                with (
                    tc.tile_pool(name="obp", bufs=3) as obp,
                    tc.tile_pool(name="wpsum", bufs=2, space="PSUM") as wpsum,
                ):
                    for c in range(NC_):
                        cs = slice(CW * c, CW * (c + 1))
                        for ht in range(KB):
                            ps_o = wpsum.tile([128, CW], f32, tag="o")
                            for h in range(H_PER_CORE):
                                nc.tensor.matmul(
                                    ps_o,
                                    wo_sb[:, h, 128 * ht : 128 * (ht + 1)],
                                    ctxa[:, h, cs],
                                    start=(h == 0),
                                    stop=(h == H_PER_CORE - 1),
                                )
                            ob = obp.tile([128, CW], bf16, tag="ob")
                            nc.vector.tensor_copy(ob, ps_o)
                            nc.sync.dma_start(
                                out_t.ap()[128 * ht : 128 * (ht + 1), cs], ob
                            )

    nc.finalize()
    return nc


_PROGRAM = None


def _get_program():
    global _PROGRAM
    if _PROGRAM is None:
        _PROGRAM = _build_program()
    return _PROGRAM


def _host_inputs(hidden_states, position_ids, wq_a, q_a_ln_w, wq_b, wkv_a,
                 kv_a_ln_w, wkv_b, wo):
    """Build the 8 per-core input maps."""
    hs = np.asarray(hidden_states, np.float32)[0]          # [S, HID]
    pos = np.asarray(position_ids)[0].astype(np.int64)     # [S]

    # rope tables (fp32, matching the reference)
    inv_freq = (1.0 / (THETA ** (np.arange(0, ROPE, 2, dtype=np.float32) / ROPE))).astype(np.float32)
    t = pos.astype(np.float32)
    freqs = np.outer(t, inv_freq).astype(np.float32)       # [S, 32]
    emb = np.concatenate([freqs, freqs], -1)               # [S, 64]
    cos = np.cos(emb).astype(np.float32)
    sin = np.sin(emb).astype(np.float32)
    cosT = np.ascontiguousarray(cos.T)                     # [64, S]
    sinT = np.ascontiguousarray(sin.T)
    sinTn = sinT.copy()
    sinTn[:32] = -sinTn[:32]                               # fold rotate_half sign
    cos2 = np.concatenate([cosT, cosT], 0)                 # [128, S]
    sin2n = np.concatenate([sinTn, sinTn], 0)

    perm = np.concatenate([np.arange(0, ROPE, 2), np.arange(1, ROPE, 2)])  # interleave

    # swap-halves permutation matrix (two independent 64 blocks)
    swapp = np.zeros((128, 128), np.float32)
    for m in range(128):
        base = (m // 64) * 64
        i = m % 64
        swapp[base + (i + 32) % 64, m] = 1.0

    maskt = np.triu(np.ones((128, 128), np.float32))

    wq_b = np.asarray(wq_b, np.float32) * np.asarray(q_a_ln_w, np.float32)[None, :]
    kvb = np.asarray(wkv_b, np.float32).reshape(16, NOPE + VD, KVL)
    wkv_a = np.asarray(wkv_a, np.float32)
    wkv_rows = np.concatenate(
        [wkv_a[:KVL], wkv_a[KVL:][perm], wkv_a[KVL:][perm]], 0
    )                                                      # [640, HID]

    shared = {
        "hid_t": _bf16(hs.T),
        "wqa_t": _bf16(np.asarray(wq_a, np.float32).T),
        "wkv_t": _bf16(wkv_rows.T),
        "kvln": _tf32_rne(np.asarray(kv_a_ln_w, np.float32)[None, :]),
        "cos2": cos2, "sin2n": sin2n,
        "swapp": _tf32_rne(swapp), "maskt": maskt,
    }

    wo = np.asarray(wo, np.float32)
    in_maps = []
    for core in range(N_CORES):
        h0 = H_PER_CORE * core
        blocks = []
        pe_rows = []
        for h in (h0, h0 + 1):
            blk = wq_b[192 * h : 192 * (h + 1)]
            blocks.append(blk[:NOPE])
            pe_rows.append(blk[NOPE:][perm])
        wqb_re = np.concatenate(blocks + pe_rows, 0)       # [384, QLR]
        wukt = np.stack(
            [np.ascontiguousarray(kvb[h, :NOPE, :].T) for h in (h0, h0 + 1)]
        )                                                  # [2, 512, 128]
        wuv2 = np.concatenate(
            [kvb[h, NOPE:, :].T for h in (h0, h0 + 1)], axis=1
        )                                                  # [512, 256]
        wo_c = np.ascontiguousarray(wo[:, VD * h0 : VD * (h0 + 2)].T)   # [256, HID]
        in_maps.append({
            **shared,
            "wqb_t": _bf16(wqb_re.T),
            "wukt": _tf32_rne(wukt),
            "wuv2": _tf32_rne(np.ascontiguousarray(wuv2)),
            "wo_t": _tf32_rne(wo_c),
        })
    return in_maps


def kernel(**inputs):
    from concourse.bass_utils import run_bass_kernel_spmd

    nc = _get_program()
    in_maps = _host_inputs(**inputs)
    res = run_bass_kernel_spmd(nc, in_maps, core_ids=list(range(N_CORES)))
    acc = None
    for r in res.results:
        o = np.asarray(r["out_t"], dtype=np.float32)
        acc = o if acc is None else acc + o
    out = np.ascontiguousarray(acc.T)[None]                # [1, S, HID]
    return out.astype(np.float32)
